# revision 5
# baseline (speedup 1.0000x reference)
"""AttnGCN (2-layer GATv2 + BN + FC) Trainium2 kernel, 8-core SPMD — v2.

Target-node sharding with degree-balanced node permutation: nodes are
dealt snake-wise by in-degree into 128-node target blocks so every block
holds ~E/nblocks edges, minimizing group padding (gpb) and balancing the
8 cores.  Edge pipeline is bf16 end-to-end (gathers, transposes, matmuls,
one-hots, AllGather); PSUM accumulation stays fp32.  Softmax is one-pass
(no max shift).  alpha = att . lrelu(z) is computed with two fused
tensor_tensor_reduce ops.  Activations use Prelu (parametric_relu), which
shares an activation table with Exp, eliminating table reloads.
Layer 1 folds the edge-weight term into the per-edge matmul via an
[x5, ew, 1, 0] lhsT layout.
"""

import sys

import numpy as np
import ml_dtypes

sys.path.insert(0, "/opt/trn_rl_repo")

BF16 = ml_dtypes.bfloat16

N = 100000
H, C = 2, 128
IN, HID, OUT = 5, 128, 5
HC = H * C

BLK = 128      # target nodes per block
GRP = 128      # edges per group
GSG = 8        # groups per supergroup
PAD_TGT = 200.0


def _balance_perm(deg_all, nb_total):
    """Snake-deal nodes (sorted by degree desc) into nb_total blocks.
    Returns new2old [nb_total*BLK] node permutation."""
    order = np.argsort(-deg_all, kind="stable")
    rounds = order.reshape(BLK, nb_total).copy()
    rounds[1::2] = rounds[1::2, ::-1]
    return np.ascontiguousarray(rounds.T.reshape(-1))  # [b*BLK + r] -> old id


def _host_prep(h, edge_index, edge_weight, n, cores):
    nb_total = -(-((n + BLK - 1) // BLK) // cores) * cores
    bpc = nb_total // cores
    npad = nb_total * BLK

    src0 = edge_index[0].astype(np.int64)
    tgt0 = edge_index[1].astype(np.int64)
    ew0 = edge_weight[:, 0].astype(np.float32)
    ew_mean = np.float32(edge_weight.astype(np.float32).mean())

    # in-degree (incl self-loop) per old node id; pads get their own
    # zero-weight self-loop too (keeps every softmax denominator > 0)
    deg_all = np.ones(npad, np.int64)
    deg_all[:n] += np.bincount(tgt0, minlength=n)

    new2old = _balance_perm(deg_all, nb_total)
    # refinement: swap nodes between over/under-full blocks until every
    # block holds <= 9*GRP edges (keeps groups-per-block at 9)
    cap = 9 * GRP
    blkdeg = deg_all[new2old].reshape(nb_total, BLK)
    bc = blkdeg.sum(axis=1)
    for _ in range(2000):
        hi = int(bc.argmax())
        if bc[hi] <= cap:
            break
        lo = int(bc.argmin())
        delta = int(bc[hi]) - (int(bc[lo]) + int(bc[hi])) // 2
        du = blkdeg[hi]
        dv = blkdeg[lo]
        best = None
        for iu in range(BLK):
            need = du[iu] - delta
            iv = int(np.abs(dv - need).argmin())
            gain = du[iu] - dv[iv]
            if 0 < gain and (best is None or
                             abs(gain - delta) < abs(best[2] - delta)):
                best = (iu, iv, gain)
        if best is None:
            break
        iu, iv, gain = best
        u, v = new2old[hi * BLK + iu], new2old[lo * BLK + iv]
        new2old[hi * BLK + iu], new2old[lo * BLK + iv] = v, u
        blkdeg[hi, iu], blkdeg[lo, iv] = dv[iv], du[iu]
        bc[hi] -= gain
        bc[lo] += gain
    old2new = np.empty(npad, np.int64)
    old2new[new2old] = np.arange(npad)

    # remapped edges + self loops for every (new) node id
    allnew = np.arange(npad)
    loop_ew = np.where(new2old < n, ew_mean, np.float32(0.0)).astype(np.float32)
    src = np.concatenate([old2new[src0], allnew])
    tgt = np.concatenate([old2new[tgt0], allnew])
    ew = np.concatenate([ew0, loop_ew])

    order = np.argsort(tgt, kind="stable")
    src, tgt, ew = src[order], tgt[order], ew[order]

    counts = np.bincount(tgt // BLK, minlength=nb_total)
    gpb = int((counts.max() + GRP - 1) // GRP)
    gpc = bpc * gpb
    gpc_pad = -(-gpc // GSG) * GSG
    extra = gpc_pad - gpc
    nsg = gpc_pad // GSG

    # group map (same for every core): (block, first, last)
    gmap = []
    for b in range(bpc):
        ng = gpb + (extra if b == bpc - 1 else 0)
        for k in range(ng):
            gmap.append((b, k == 0, k == ng - 1))
    assert len(gmap) == gpc_pad

    blk_starts = np.zeros(nb_total + 1, np.int64)
    np.cumsum(counts, out=blk_starts[1:])

    nodedeg = np.maximum(np.bincount(tgt, minlength=npad), 1).astype(np.float32)
    invd_all = (1.0 / nodedeg).astype(np.float32)

    # permuted h: row newid = h[old] if old is real else 0
    hperm = np.zeros((npad, IN), np.float32)
    valid = new2old < n
    hperm[valid] = h.astype(np.float32)[new2old[valid]]

    per_core = []
    for c in range(cores):
        IDX = np.zeros((gpc_pad, GRP), np.int32)
        TGL = np.full((gpc_pad, GRP), PAD_TGT, np.float32)
        EWG = np.zeros((gpc_pad, GRP), np.float32)
        for b in range(c * bpc, (c + 1) * bpc):
            bl = b - c * bpc
            s, e = int(blk_starts[b]), int(blk_starts[b + 1])
            m = e - s
            assert m <= gpb * GRP, (m, gpb * GRP)
            fi = np.arange(bl * gpb * GRP, bl * gpb * GRP + m)
            IDX.reshape(-1)[fi] = src[s:e]
            TGL.reshape(-1)[fi] = (tgt[s:e] - b * BLK).astype(np.float32)
            EWG.reshape(-1)[fi] = ew[s:e]

        idxt = np.zeros((nsg, GRP, GSG), np.int32)
        tgtc = np.zeros((nsg, GRP, GSG), np.float32)
        tgtr = np.zeros((nsg, 1, GSG * GRP), BF16)
        ewt = np.zeros((nsg, 2, GSG * GRP), BF16)
        ewc1 = np.zeros((nsg, GRP, GSG, 3), BF16)
        for sg in range(nsg):
            for j in range(GSG):
                g = sg * GSG + j
                idxt[sg, :, j] = IDX[g]
                tgtc[sg, :, j] = TGL[g]
                tgtr[sg, 0, j * GRP:(j + 1) * GRP] = TGL[g].astype(BF16)
                ewt[sg, 0, j * GRP:(j + 1) * GRP] = EWG[g].astype(BF16)
                ewc1[sg, :, j, 0] = EWG[g].astype(BF16)
        ewt[:, 1, :] = 1.0
        ewc1[:, :, :, 1] = 1.0

        invd = invd_all[c * bpc * BLK:(c + 1) * bpc * BLK]
        per_core.append(dict(
            IDXT=idxt, TGTC=tgtc, TGTR=tgtr, EWT=ewt, EWC1=ewc1,
            INVC=np.ascontiguousarray(invd.reshape(bpc, BLK, 1)),
            INVR=np.ascontiguousarray(
                invd.reshape(bpc, 1, BLK).astype(BF16)),
            HTC=np.ascontiguousarray(
                hperm[c * bpc * BLK:(c + 1) * bpc * BLK].T.astype(BF16)),
        ))
    h5p = np.zeros((npad, 128), BF16)
    h5p[:, :IN] = hperm.astype(BF16)
    return per_core, h5p, gmap, bpc, nsg, npad, new2old


def _weights_host(p):
    eps = np.float32(1e-5)

    def affine(g, b, m, v, bias):
        s = (np.asarray(g) / np.sqrt(np.asarray(v) + eps)).astype(np.float32)
        bb = (np.asarray(b) + (np.asarray(bias) - np.asarray(m)) * s).astype(np.float32)
        return s, bb

    s1, b1 = affine(p["bn1g"], p["bn1b"], p["bn1m"], p["bn1v"], p["bias1"])
    s2, b2 = affine(p["bn2g"], p["bn2b"], p["bn2m"], p["bn2v"], p["bias2"])
    att1 = np.asarray(p["att1"], np.float32).reshape(-1)
    att2 = np.asarray(p["att2"], np.float32).reshape(-1)
    wl1 = np.asarray(p["Wl1"], np.float32)
    wl2 = np.asarray(p["Wl2"], np.float32)
    bl1 = np.asarray(p["bl1"], np.float32)
    bl2 = np.asarray(p["bl2"], np.float32)

    # layer-1 fused per-edge weight: rows [Wl1(5); We1; bl1+br1; 0]
    wl1f = np.zeros((8, HC), np.float32)
    wl1f[0:IN] = wl1
    wl1f[IN] = np.asarray(p["We1"], np.float32)[0]
    wl1f[IN + 1] = bl1 + np.asarray(p["br1"], np.float32)

    bf = lambda a: np.asarray(a).astype(BF16)
    return dict(
        WL1F=bf(wl1f),
        WL1P=bf(wl1), WL2P=bf(wl2),
        WR1=bf(p["Wr1"]), WR2=bf(p["Wr2"]),
        WL2A=bf(wl2),
        WB2=bf(np.stack([np.asarray(p["We2"], np.float32)[0],
                         bl2 + np.asarray(p["br2"], np.float32)])),
        ATT1=bf(np.tile(att1[None, :], (128, 1))),
        ATT2=bf(np.tile(att2[None, :], (128, 1))),
        SM1=s1[:, None].astype(np.float32), BM1=b1[:, None].astype(np.float32),
        SM2=s2[:, None].astype(np.float32), BM2=b2[:, None].astype(np.float32),
        BL1=bf((0.5 * (bl1[:C] + bl1[C:]))[None, :]),
        BL2=bf((0.5 * (bl2[:C] + bl2[C:]))[None, :]),
        FCW=bf(p["fcw"]),
        FCB=np.asarray(p["fcb"], np.float32)[:, None],
    )


def _build_nc(gmap, bpc, nsg, npad, cores):
    import concourse.bass as bass
    import concourse.bacc as bacc
    import concourse.mybir as mybir
    import concourse.tile as tile
    from concourse.masks import make_identity
    from contextlib import ExitStack

    dt = mybir.dt
    AF = mybir.ActivationFunctionType
    ALU = mybir.AluOpType
    nsl = bpc * BLK
    gpc_pad = len(gmap)

    nc = bacc.Bacc()
    bf = dt.bfloat16

    IDXT = nc.dram_tensor("IDXT", [nsg, GRP, GSG], dt.int32, kind="ExternalInput")
    TGTC = nc.dram_tensor("TGTC", [nsg, GRP, GSG], dt.float32, kind="ExternalInput")
    TGTR = nc.dram_tensor("TGTR", [nsg, 1, GSG * GRP], bf, kind="ExternalInput")
    EWT = nc.dram_tensor("EWT", [nsg, 2, GSG * GRP], bf, kind="ExternalInput")
    EWC1 = nc.dram_tensor("EWC1", [nsg, GRP, GSG, 3], bf, kind="ExternalInput")
    INVC = nc.dram_tensor("INVC", [bpc, BLK, 1], dt.float32, kind="ExternalInput")
    INVR = nc.dram_tensor("INVR", [bpc, 1, BLK], bf, kind="ExternalInput")
    H5 = nc.dram_tensor("H5", [npad, 128], bf, kind="ExternalInput")
    HTC = nc.dram_tensor("HTC", [IN, nsl], bf, kind="ExternalInput")

    CONST_SHAPES = [
        ("WL1F", [8, HC], bf), ("WL1P", [IN, HC], bf), ("WL2P", [HID, HC], bf),
        ("WR1", [IN, HC], bf), ("WR2", [HID, HC], bf),
        ("WL2A", [HID, HC], bf), ("WB2", [2, HC], bf),
        ("ATT1", [128, HC], bf), ("ATT2", [128, HC], bf),
        ("SM1", [C, 1], dt.float32), ("BM1", [C, 1], dt.float32),
        ("SM2", [C, 1], dt.float32), ("BM2", [C, 1], dt.float32),
        ("BL1", [1, C], bf), ("BL2", [1, C], bf),
        ("FCW", [HID, OUT], bf), ("FCB", [OUT, 1], dt.float32),
    ]
    CONSTS = {nm: nc.dram_tensor(nm, sh, d, kind="ExternalInput")
              for nm, sh, d in CONST_SHAPES}
    OUTT = nc.dram_tensor("OUTT", [OUT, nsl], dt.float32, kind="ExternalOutput")

    with ExitStack() as ctx:
        tc = ctx.enter_context(tile.TileContext(nc))
        cpool = ctx.enter_context(tc.tile_pool(name="consts", bufs=1))
        spool = ctx.enter_context(tc.tile_pool(name="sg", bufs=4))
        gpool = ctx.enter_context(tc.tile_pool(name="grp", bufs=8))
        bpool = ctx.enter_context(tc.tile_pool(name="blk", bufs=4))
        dpool = ctx.enter_context(tc.tile_pool(name="dram", bufs=1, space="DRAM"))
        pt = ctx.enter_context(tc.tile_pool(name="pt", bufs=3, space="PSUM"))
        ps = ctx.enter_context(tc.tile_pool(name="ps", bufs=2, space="PSUM"))
        px = ctx.enter_context(tc.tile_pool(name="px", bufs=1, space="PSUM"))
        pxt = ctx.enter_context(tc.tile_pool(name="pxt", bufs=2, space="PSUM"))

        def mixtile():
            return px.tile([128, 512], dt.float32, tag="mix", name="mix")

        ident = cpool.tile([128, 128], bf)
        make_identity(nc, ident[:])
        iota_i = cpool.tile([128, 128], dt.int32)
        nc.gpsimd.iota(iota_i[:], pattern=[[1, 128]], base=0, channel_multiplier=0)
        iota_m = cpool.tile([128, 128], bf)
        nc.vector.tensor_copy(iota_m[:], iota_i[:])
        ones_r = cpool.tile([1, 128], bf)
        nc.gpsimd.memset(ones_r[:], 1.0)
        iotap_i = cpool.tile([128, 1], dt.int32)
        nc.gpsimd.iota(iotap_i[:], pattern=[[1, 1]], base=0, channel_multiplier=1)
        iota_p = cpool.tile([128, 1], dt.float32)
        nc.vector.tensor_copy(iota_p[:], iotap_i[:])

        cs = {}
        for nm, t in CONSTS.items():
            til = cpool.tile(list(t.shape), t.dtype, name=f"c_{nm}")
            nc.sync.dma_start(out=til[:], in_=t[:, :])
            cs[nm] = til

        YS = dpool.tile([nsl, HID], bf, name="YS")
        YST = dpool.tile([HID, nsl], bf, name="YST")
        YF = dpool.tile([npad, HID], bf, name="YF")

        def epilogue(li, b, s_t):
            D = IN if li == 1 else HID
            DEN = IN + 1 if li == 1 else HID  # denominator column in s
            wlp = cs["WL1P"] if li == 1 else cs["WL2P"]
            sm = cs["SM1"] if li == 1 else cs["SM2"]
            bm = cs["BM1"] if li == 1 else cs["BM2"]
            blv = cs["BL1"] if li == 1 else cs["BL2"]
            invc = bpool.tile([BLK, 1], dt.float32, tag="invc")
            nc.sync.dma_start(out=invc[:], in_=INVC[b])
            invr = bpool.tile([1, BLK], bf, tag="invr")
            nc.sync.dma_start(out=invr[:], in_=INVR[b])
            cf = bpool.tile([BLK, 2 * 128], bf, tag="cf")
            for hh in range(H):
                rec = bpool.tile([BLK, 1], dt.float32, tag=f"rec{hh}")
                nc.vector.reciprocal(rec[:], s_t[hh][:, DEN:DEN + 1])
                f = bpool.tile([BLK, 1], dt.float32, tag=f"f{hh}")
                nc.vector.tensor_scalar(
                    out=f[:], in0=rec[:], scalar1=invc[:], scalar2=0.5,
                    op0=ALU.mult, op1=ALU.mult)
                nc.vector.tensor_scalar(
                    out=cf[:, hh * 128:(hh + 1) * 128], in0=s_t[hh][:, 0:128],
                    scalar1=f[:], scalar2=None, op0=ALU.mult)
            cft = bpool.tile([128, 2 * BLK], bf, tag="cft")
            for hh in range(H):
                nc.sync.dma_start_transpose(
                    out=cft[:, hh * BLK:(hh + 1) * BLK],
                    in_=cf[:, hh * 128:(hh + 1) * 128])
            qt_ps = mixtile()[:, 0:BLK]
            for hh in range(H):
                nc.tensor.matmul(out=qt_ps[:],
                                 lhsT=wlp[0:D, hh * C:(hh + 1) * C],
                                 rhs=cft[0:D, hh * BLK:(hh + 1) * BLK],
                                 start=(hh == 0), stop=False,
                                 skip_group_check=True)
            nc.tensor.matmul(out=qt_ps[:], lhsT=blv[:], rhs=invr[:],
                             start=False, stop=True, skip_group_check=True)
            yt = bpool.tile([C, BLK], bf, tag="yt")
            nc.scalar.activation(yt[:], qt_ps[:], AF.Prelu,
                                 bias=bm[:], scale=sm[:], alpha=0.01)
            if li == 1:
                ysb = bpool.tile([BLK, C], bf, tag="ysb")
                nc.sync.dma_start_transpose(out=ysb[:], in_=yt[:])
                nc.sync.dma_start(out=YS[b * BLK:(b + 1) * BLK, :], in_=ysb[:])
                nc.sync.dma_start(out=YST[:, b * BLK:(b + 1) * BLK], in_=yt[:])
            else:
                o_ps = mixtile()[0:OUT, 0:BLK]
                nc.tensor.matmul(out=o_ps[:], lhsT=cs["FCW"][:],
                                 rhs=yt[:], start=True, stop=True,
                                 skip_group_check=True)
                osb = bpool.tile([OUT, BLK], dt.float32, tag="osb")
                nc.vector.tensor_scalar(out=osb[:], in0=o_ps[:],
                                        scalar1=cs["FCB"][:], scalar2=None,
                                        op0=ALU.add)
                nc.sync.dma_start(out=OUTT[:, b * BLK:(b + 1) * BLK], in_=osb[:])

        def build_xrb(li, b):
            wr = cs["WR1"] if li == 1 else cs["WR2"]
            if li == 1:
                xl_ = bpool.tile([IN, BLK], bf, tag="xrl")
                nc.sync.dma_start(out=xl_[:], in_=HTC[:, b * BLK:(b + 1) * BLK])
                lhs = xl_
            else:
                ytb = bpool.tile([HID, BLK], bf, tag="ytb")
                nc.sync.dma_start(out=ytb[:], in_=YST[:, b * BLK:(b + 1) * BLK])
                lhs = ytb
            xr_ps = mixtile()[:, 0:HC]
            nc.tensor.matmul(out=xr_ps[:], lhsT=lhs[:], rhs=wr[:],
                             start=True, stop=True, skip_group_check=True)
            xrb = bpool.tile([BLK, HC], bf, tag="xrb")
            nc.vector.tensor_copy(xrb[:], xr_ps[:])
            return xrb

        def layer(li):
            D = 128                              # gathered feature cols
            TD = 8 if li == 1 else HID          # transpose rows
            RW = 128 if li == 1 else 132        # r8 width
            AGW = 128 if li == 1 else HID + 1   # agg rhs width (incl den)
            att = cs["ATT1"] if li == 1 else cs["ATT2"]
            gsrc = H5[:, :] if li == 1 else YF[:, :]

            xrb_of = {}
            s_of = {}
            pending = []

            def pass2(sg, r8, tgc8, ex8):
                for j in range(GSG):
                    g = sg * GSG + j
                    b, first, last = gmap[g]
                    if first:
                        sfull = ps.tile([BLK, 512], dt.float32, tag="s",
                                        name=f"s_{li}_{b}")
                        s_of[b] = [sfull[:, hh * 256:hh * 256 + AGW]
                                   for hh in range(H)]
                    for hh in range(H):
                        oh = gpool.tile([GRP, BLK], bf, tag=f"oh{hh}",
                                        name="oh")
                        gi = j * H + hh
                        nc.vector.tensor_scalar(
                            out=oh[:], in0=iota_m[:], scalar1=tgc8[:, j:j + 1],
                            scalar2=ex8[:, gi:gi + 1],
                            op0=ALU.is_equal, op1=ALU.mult)
                        nc.tensor.matmul(
                            out=s_of[b][hh], lhsT=oh[:],
                            rhs=r8[:, j, 0:AGW],
                            start=(first and hh == 0),
                            stop=(last and hh == H - 1),
                            skip_group_check=True)
                    if last:
                        epilogue(li, b, s_of.pop(b))
                        xrb_of.pop(b, None)

            for sg in range(nsg):
                idx8 = spool.tile([GRP, GSG], dt.int32, tag="idx8")
                nc.sync.dma_start(out=idx8[:], in_=IDXT[sg])
                tgc8 = spool.tile([GRP, GSG], dt.float32, tag="tgc8")
                nc.sync.dma_start(out=tgc8[:], in_=TGTC[sg])
                tgbc = spool.tile([BLK, GSG * GRP], bf, tag="tgbc")
                nc.sync.dma_start(
                    out=tgbc[:],
                    in_=TGTR[sg].squeeze().partition_broadcast(BLK))
                ew8 = spool.tile([2, GSG * GRP], bf, tag="ew8")
                nc.sync.dma_start(out=ew8[:], in_=EWT[sg])

                r8 = spool.tile([GRP, GSG, RW], bf, tag=f"r8_{li}", name="r8")
                for jg in range(GSG):
                    nc.gpsimd.indirect_dma_start(
                        out=r8[:, jg, 0:D], out_offset=None, in_=gsrc,
                        in_offset=bass.IndirectOffsetOnAxis(
                            ap=idx8[:, jg:jg + 1], axis=0))
                if li == 1:
                    # overwrite cols 5:8 with (ew, 1, 0) after the gather
                    nc.sync.dma_start(out=r8[:, :, IN:IN + 3], in_=EWC1[sg])
                else:
                    nc.gpsimd.memset(r8[:, :, HID:HID + 1], 1.0)

                ot8 = spool.tile([BLK, GSG * GRP], bf, tag="ot8")
                nc.vector.tensor_scalar(
                    out=ot8[:, :], in0=tgbc[:],
                    scalar1=iota_p[:], scalar2=None, op0=ALU.is_equal)

                alph8 = spool.tile([GRP, GSG * H], dt.float32, tag="alph8")
                ex8 = spool.tile([GRP, GSG * H], dt.float32, tag="ex8")

                xt2s = []
                for j in range(0, GSG, 2):
                    xt_ps = pxt.tile([HID, 2 * GRP], bf, tag="xt", name="xt_ps")
                    for jj in (0, 1):
                        nc.tensor.transpose(
                            out=xt_ps[:, jj * GRP:(jj + 1) * GRP],
                            in_=r8[:, j + jj, 0:128], identity=ident[:])
                    xt2p = gpool.tile([HID, 2 * GRP], bf, tag="xt2", name="xt2")
                    nc.scalar.copy(xt2p[:], xt_ps[:])
                    xt2s.append(xt2p[:, 0:GRP])
                    xt2s.append(xt2p[:, GRP:2 * GRP])
                for j in range(GSG):
                    g = sg * GSG + j
                    b, first, last = gmap[g]
                    if first:
                        xrb_of[b] = build_xrb(li, b)
                    xt2 = xt2s[j]
                    esl = slice(j * GRP, (j + 1) * GRP)
                    t_ps = pt.tile([GRP, HC], dt.float32, tag="t", name="t")
                    if li == 1:
                        nc.tensor.matmul(out=t_ps[:], lhsT=xt2[0:8, :],
                                         rhs=cs["WL1F"][:], start=True,
                                         stop=False, skip_group_check=True)
                        nc.tensor.matmul(out=t_ps[:], lhsT=ot8[:, esl],
                                         rhs=xrb_of[b][:], start=False,
                                         stop=True, skip_group_check=True)
                    else:
                        nc.tensor.matmul(out=t_ps[:], lhsT=xt2[:, :],
                                         rhs=cs["WL2A"][:], start=True,
                                         stop=False, skip_group_check=True)
                        nc.tensor.matmul(out=t_ps[:], lhsT=ot8[:, esl],
                                         rhs=xrb_of[b][:], start=False,
                                         stop=False, skip_group_check=True)
                        nc.tensor.matmul(out=t_ps[:], lhsT=ew8[:, esl],
                                         rhs=cs["WB2"][:], start=False,
                                         stop=True, skip_group_check=True)
                    zl = gpool.tile([GRP, HC], bf, tag="zl")
                    nc.scalar.activation(zl[:], t_ps[:], AF.Prelu, alpha=0.2)
                    za = gpool.tile([GRP, H, C], bf, tag="za")
                    nc.vector.tensor_mul(za[:, :, :], zl[:].rearrange(
                        "p (h c) -> p h c", h=H), att[:].rearrange(
                        "p (h c) -> p h c", h=H))
                    nc.vector.tensor_reduce(
                        out=alph8[:, j * H:(j + 1) * H], in_=za[:, :, :],
                        axis=mybir.AxisListType.X, op=ALU.add)

                nc.scalar.activation(ex8[:], alph8[:], AF.Exp)

                pending.append((sg, r8, tgc8, ex8))
                if len(pending) > 1:
                    pass2(*pending.pop(0))

            for args in pending:
                pass2(*args)
            pending.clear()

        layer(1)
        if cores > 1:
            nc.gpsimd.collective_compute(
                "AllGather", mybir.AluOpType.bypass,
                replica_groups=[list(range(cores))],
                ins=[YS[:, :]], outs=[YF[:, :]])
        else:
            nc.sync.dma_start(out=YF[:, :], in_=YS[:, :])
        layer(2)

    nc.compile()
    return nc


def run(inputs, n, cores, run_sim=False, trace=False):
    h = np.asarray(inputs["h"], np.float32)
    edge_index = np.asarray(inputs["edge_index"])
    edge_weight = np.asarray(inputs["edge_weight"], np.float32)

    per_core, h5p, gmap, bpc, nsg, npad, new2old = _host_prep(
        h, edge_index, edge_weight, n, cores)
    consts = _weights_host(inputs)
    consts["H5"] = h5p

    nc = _build_nc(gmap, bpc, nsg, npad, cores)

    in_maps = []
    for c in range(cores):
        m = dict(consts)
        m.update(per_core[c])
        in_maps.append({k: np.ascontiguousarray(v) for k, v in m.items()})

    if run_sim:
        from concourse import bass_interp
        if cores == 1:
            sim = bass_interp.CoreSim(nc)
            sims = [sim]
        else:
            sim = bass_interp.MultiCoreSim(nc, num_cores=cores)
            sims = list(sim.cores.values())
        for ci, cs_ in enumerate(sims):
            for k, v in in_maps[ci].items():
                cs_.tensor(k)[:] = v
        sim.simulate()
        outs = [np.array(cs_.tensor("OUTT")).T for cs_ in sims]
        res = None
    else:
        from concourse.bass_utils import run_bass_kernel_spmd
        import tempfile
        kw = {}
        if trace:
            kw = dict(trace=True, tmpdir=tempfile.mkdtemp(prefix="basstrace_"))
        res = run_bass_kernel_spmd(nc, in_maps, core_ids=list(range(cores)), **kw)
        outs = [r["OUTT"].T for r in res.results]

    out_perm = np.concatenate(outs, axis=0)  # [npad, OUT] in permuted order
    out = np.empty((n, OUT), np.float32)
    valid = new2old < n
    out[new2old[valid]] = out_perm[valid].astype(np.float32)
    return out, res


def kernel(**inputs):
    out, _ = run(inputs, N, 8)
    return out


# revision 6
# speedup vs baseline: 1.0405x; 1.0405x over previous
"""AttnGCN (2-layer GATv2 + BN + FC) Trainium2 kernel, 8-core SPMD — v2.

Target-node sharding with degree-balanced node permutation: nodes are
dealt snake-wise by in-degree into 128-node target blocks so every block
holds ~E/nblocks edges, minimizing group padding (gpb) and balancing the
8 cores.  Edge pipeline is bf16 end-to-end (gathers, transposes, matmuls,
one-hots, AllGather); PSUM accumulation stays fp32.  Softmax is one-pass
(no max shift).  alpha = att . lrelu(z) is computed with two fused
tensor_tensor_reduce ops.  Activations use Prelu (parametric_relu), which
shares an activation table with Exp, eliminating table reloads.
Layer 1 folds the edge-weight term into the per-edge matmul via an
[x5, ew, 1, 0] lhsT layout.
"""

import sys

import numpy as np
import ml_dtypes

sys.path.insert(0, "/opt/trn_rl_repo")

BF16 = ml_dtypes.bfloat16

N = 100000
H, C = 2, 128
IN, HID, OUT = 5, 128, 5
HC = H * C

BLK = 128      # target nodes per block
GRP = 128      # edges per group
GSG = 8        # groups per supergroup
PAD_TGT = 200.0


def _balance_perm(deg_all, nb_total):
    """Snake-deal nodes (sorted by degree desc) into nb_total blocks.
    Returns new2old [nb_total*BLK] node permutation."""
    order = np.argsort(-deg_all, kind="stable")
    rounds = order.reshape(BLK, nb_total).copy()
    rounds[1::2] = rounds[1::2, ::-1]
    return np.ascontiguousarray(rounds.T.reshape(-1))  # [b*BLK + r] -> old id


def _host_prep(h, edge_index, edge_weight, n, cores):
    nb_total = -(-((n + BLK - 1) // BLK) // cores) * cores
    bpc = nb_total // cores
    npad = nb_total * BLK

    src0 = edge_index[0].astype(np.int64)
    tgt0 = edge_index[1].astype(np.int64)
    ew0 = edge_weight[:, 0].astype(np.float32)
    ew_mean = np.float32(edge_weight.astype(np.float32).mean())

    # in-degree (incl self-loop) per old node id; pads get their own
    # zero-weight self-loop too (keeps every softmax denominator > 0)
    deg_all = np.ones(npad, np.int64)
    deg_all[:n] += np.bincount(tgt0, minlength=n)

    new2old = _balance_perm(deg_all, nb_total)
    # refinement: swap nodes between over/under-full blocks until every
    # block holds <= 9*GRP edges (keeps groups-per-block at 9)
    cap = 9 * GRP
    blkdeg = deg_all[new2old].reshape(nb_total, BLK)
    bc = blkdeg.sum(axis=1)
    for _ in range(2000):
        hi = int(bc.argmax())
        if bc[hi] <= cap:
            break
        lo = int(bc.argmin())
        delta = int(bc[hi]) - (int(bc[lo]) + int(bc[hi])) // 2
        du = blkdeg[hi]
        dv = blkdeg[lo]
        best = None
        for iu in range(BLK):
            need = du[iu] - delta
            iv = int(np.abs(dv - need).argmin())
            gain = du[iu] - dv[iv]
            if 0 < gain and (best is None or
                             abs(gain - delta) < abs(best[2] - delta)):
                best = (iu, iv, gain)
        if best is None:
            break
        iu, iv, gain = best
        u, v = new2old[hi * BLK + iu], new2old[lo * BLK + iv]
        new2old[hi * BLK + iu], new2old[lo * BLK + iv] = v, u
        blkdeg[hi, iu], blkdeg[lo, iv] = dv[iv], du[iu]
        bc[hi] -= gain
        bc[lo] += gain
    old2new = np.empty(npad, np.int64)
    old2new[new2old] = np.arange(npad)

    # remapped edges + self loops for every (new) node id
    allnew = np.arange(npad)
    loop_ew = np.where(new2old < n, ew_mean, np.float32(0.0)).astype(np.float32)
    src = np.concatenate([old2new[src0], allnew])
    tgt = np.concatenate([old2new[tgt0], allnew])
    ew = np.concatenate([ew0, loop_ew])

    order = np.argsort(tgt, kind="stable")
    src, tgt, ew = src[order], tgt[order], ew[order]

    counts = np.bincount(tgt // BLK, minlength=nb_total)
    gpb = int((counts.max() + GRP - 1) // GRP)
    gpc = bpc * gpb
    gpc_pad = -(-gpc // GSG) * GSG
    extra = gpc_pad - gpc
    nsg = gpc_pad // GSG

    # group map (same for every core): (block, first, last)
    gmap = []
    for b in range(bpc):
        ng = gpb + (extra if b == bpc - 1 else 0)
        for k in range(ng):
            gmap.append((b, k == 0, k == ng - 1))
    assert len(gmap) == gpc_pad

    blk_starts = np.zeros(nb_total + 1, np.int64)
    np.cumsum(counts, out=blk_starts[1:])

    nodedeg = np.maximum(np.bincount(tgt, minlength=npad), 1).astype(np.float32)
    invd_all = (1.0 / nodedeg).astype(np.float32)

    # permuted h: row newid = h[old] if old is real else 0
    hperm = np.zeros((npad, IN), np.float32)
    valid = new2old < n
    hperm[valid] = h.astype(np.float32)[new2old[valid]]

    per_core = []
    for c in range(cores):
        IDX = np.zeros((gpc_pad, GRP), np.int32)
        TGL = np.full((gpc_pad, GRP), PAD_TGT, np.float32)
        EWG = np.zeros((gpc_pad, GRP), np.float32)
        for b in range(c * bpc, (c + 1) * bpc):
            bl = b - c * bpc
            s, e = int(blk_starts[b]), int(blk_starts[b + 1])
            m = e - s
            assert m <= gpb * GRP, (m, gpb * GRP)
            fi = np.arange(bl * gpb * GRP, bl * gpb * GRP + m)
            IDX.reshape(-1)[fi] = src[s:e]
            TGL.reshape(-1)[fi] = (tgt[s:e] - b * BLK).astype(np.float32)
            EWG.reshape(-1)[fi] = ew[s:e]

        idxt = np.zeros((nsg, GRP, GSG), np.int32)
        tgtc = np.zeros((nsg, GRP, GSG), np.float32)
        tgtr = np.zeros((nsg, 1, GSG * GRP), BF16)
        ewt = np.zeros((nsg, 2, GSG * GRP), BF16)
        ewc1 = np.zeros((nsg, GRP, GSG, 3), BF16)
        for sg in range(nsg):
            for j in range(GSG):
                g = sg * GSG + j
                idxt[sg, :, j] = IDX[g]
                tgtc[sg, :, j] = TGL[g]
                tgtr[sg, 0, j * GRP:(j + 1) * GRP] = TGL[g].astype(BF16)
                ewt[sg, 0, j * GRP:(j + 1) * GRP] = EWG[g].astype(BF16)
                ewc1[sg, :, j, 0] = EWG[g].astype(BF16)
        ewt[:, 1, :] = 1.0
        ewc1[:, :, :, 1] = 1.0

        invd = invd_all[c * bpc * BLK:(c + 1) * bpc * BLK]
        per_core.append(dict(
            IDXT=idxt, TGTC=tgtc, TGTR=tgtr, EWT=ewt, EWC1=ewc1,
            INVC=np.ascontiguousarray(invd.reshape(bpc, BLK, 1)),
            INVR=np.ascontiguousarray(
                invd.reshape(bpc, 1, BLK).astype(BF16)),
            HTC=np.ascontiguousarray(
                hperm[c * bpc * BLK:(c + 1) * bpc * BLK].T.astype(BF16)),
        ))
    h5p = np.zeros((npad, 128), BF16)
    h5p[:, :IN] = hperm.astype(BF16)
    return per_core, h5p, gmap, bpc, nsg, npad, new2old


def _weights_host(p):
    eps = np.float32(1e-5)

    def affine(g, b, m, v, bias):
        s = (np.asarray(g) / np.sqrt(np.asarray(v) + eps)).astype(np.float32)
        bb = (np.asarray(b) + (np.asarray(bias) - np.asarray(m)) * s).astype(np.float32)
        return s, bb

    s1, b1 = affine(p["bn1g"], p["bn1b"], p["bn1m"], p["bn1v"], p["bias1"])
    s2, b2 = affine(p["bn2g"], p["bn2b"], p["bn2m"], p["bn2v"], p["bias2"])
    att1 = np.asarray(p["att1"], np.float32).reshape(-1)
    att2 = np.asarray(p["att2"], np.float32).reshape(-1)
    wl1 = np.asarray(p["Wl1"], np.float32)
    wl2 = np.asarray(p["Wl2"], np.float32)
    bl1 = np.asarray(p["bl1"], np.float32)
    bl2 = np.asarray(p["bl2"], np.float32)

    # layer-1 fused per-edge weight: rows [Wl1(5); We1; bl1+br1; 0]
    wl1f = np.zeros((8, HC), np.float32)
    wl1f[0:IN] = wl1
    wl1f[IN] = np.asarray(p["We1"], np.float32)[0]
    wl1f[IN + 1] = bl1 + np.asarray(p["br1"], np.float32)

    bf = lambda a: np.asarray(a).astype(BF16)
    return dict(
        WL1F=bf(wl1f),
        WL1P=bf(wl1), WL2P=bf(wl2),
        WR1=bf(p["Wr1"]), WR2=bf(p["Wr2"]),
        WL2A=bf(wl2),
        WB2=bf(np.stack([np.asarray(p["We2"], np.float32)[0],
                         bl2 + np.asarray(p["br2"], np.float32)])),
        ATT1=bf(np.tile(att1[None, :], (128, 1))),
        ATT2=bf(np.tile(att2[None, :], (128, 1))),
        SM1=s1[:, None].astype(np.float32), BM1=b1[:, None].astype(np.float32),
        SM2=s2[:, None].astype(np.float32), BM2=b2[:, None].astype(np.float32),
        BL1=bf((0.5 * (bl1[:C] + bl1[C:]))[None, :]),
        BL2=bf((0.5 * (bl2[:C] + bl2[C:]))[None, :]),
        FCW=bf(p["fcw"]),
        FCB=np.asarray(p["fcb"], np.float32)[:, None],
    )


def _build_nc(gmap, bpc, nsg, npad, cores):
    import concourse.bass as bass
    import concourse.bacc as bacc
    import concourse.mybir as mybir
    import concourse.tile as tile
    from concourse.masks import make_identity
    from contextlib import ExitStack

    dt = mybir.dt
    AF = mybir.ActivationFunctionType
    ALU = mybir.AluOpType
    nsl = bpc * BLK
    gpc_pad = len(gmap)

    nc = bacc.Bacc()
    bf = dt.bfloat16

    IDXT = nc.dram_tensor("IDXT", [nsg, GRP, GSG], dt.int32, kind="ExternalInput")
    TGTC = nc.dram_tensor("TGTC", [nsg, GRP, GSG], dt.float32, kind="ExternalInput")
    TGTR = nc.dram_tensor("TGTR", [nsg, 1, GSG * GRP], bf, kind="ExternalInput")
    EWT = nc.dram_tensor("EWT", [nsg, 2, GSG * GRP], bf, kind="ExternalInput")
    EWC1 = nc.dram_tensor("EWC1", [nsg, GRP, GSG, 3], bf, kind="ExternalInput")
    INVC = nc.dram_tensor("INVC", [bpc, BLK, 1], dt.float32, kind="ExternalInput")
    INVR = nc.dram_tensor("INVR", [bpc, 1, BLK], bf, kind="ExternalInput")
    H5 = nc.dram_tensor("H5", [npad, 128], bf, kind="ExternalInput")
    HTC = nc.dram_tensor("HTC", [IN, nsl], bf, kind="ExternalInput")

    CONST_SHAPES = [
        ("WL1F", [8, HC], bf), ("WL1P", [IN, HC], bf), ("WL2P", [HID, HC], bf),
        ("WR1", [IN, HC], bf), ("WR2", [HID, HC], bf),
        ("WL2A", [HID, HC], bf), ("WB2", [2, HC], bf),
        ("ATT1", [128, HC], bf), ("ATT2", [128, HC], bf),
        ("SM1", [C, 1], dt.float32), ("BM1", [C, 1], dt.float32),
        ("SM2", [C, 1], dt.float32), ("BM2", [C, 1], dt.float32),
        ("BL1", [1, C], bf), ("BL2", [1, C], bf),
        ("FCW", [HID, OUT], bf), ("FCB", [OUT, 1], dt.float32),
    ]
    CONSTS = {nm: nc.dram_tensor(nm, sh, d, kind="ExternalInput")
              for nm, sh, d in CONST_SHAPES}
    OUTT = nc.dram_tensor("OUTT", [OUT, nsl], dt.float32, kind="ExternalOutput")

    with ExitStack() as ctx:
        tc = ctx.enter_context(tile.TileContext(nc))
        cpool = ctx.enter_context(tc.tile_pool(name="consts", bufs=1))
        spool = ctx.enter_context(tc.tile_pool(name="sg", bufs=4))
        gpool = ctx.enter_context(tc.tile_pool(name="grp", bufs=8))
        bpool = ctx.enter_context(tc.tile_pool(name="blk", bufs=4))
        dpool = ctx.enter_context(tc.tile_pool(name="dram", bufs=1, space="DRAM"))
        pt = ctx.enter_context(tc.tile_pool(name="pt", bufs=3, space="PSUM"))
        ps = ctx.enter_context(tc.tile_pool(name="ps", bufs=2, space="PSUM"))
        px = ctx.enter_context(tc.tile_pool(name="px", bufs=1, space="PSUM"))
        pxt = ctx.enter_context(tc.tile_pool(name="pxt", bufs=2, space="PSUM"))

        def mixtile():
            return px.tile([128, 512], dt.float32, tag="mix", name="mix")

        ident = cpool.tile([128, 128], bf)
        make_identity(nc, ident[:])
        iota_i = cpool.tile([128, 128], dt.int32)
        nc.gpsimd.iota(iota_i[:], pattern=[[1, 128]], base=0, channel_multiplier=0)
        iota_m = cpool.tile([128, 128], bf)
        nc.vector.tensor_copy(iota_m[:], iota_i[:])
        ones_r = cpool.tile([1, 128], bf)
        nc.gpsimd.memset(ones_r[:], 1.0)
        iotap_i = cpool.tile([128, 1], dt.int32)
        nc.gpsimd.iota(iotap_i[:], pattern=[[1, 1]], base=0, channel_multiplier=1)
        iota_p = cpool.tile([128, 1], dt.float32)
        nc.vector.tensor_copy(iota_p[:], iotap_i[:])

        cs = {}
        for nm, t in CONSTS.items():
            til = cpool.tile(list(t.shape), t.dtype, name=f"c_{nm}")
            nc.sync.dma_start(out=til[:], in_=t[:, :])
            cs[nm] = til

        YS = dpool.tile([nsl, HID], bf, name="YS")
        YST = dpool.tile([HID, nsl], bf, name="YST")
        YF = dpool.tile([npad, HID], bf, name="YF")

        def epilogue(li, b, s_t):
            D = IN if li == 1 else HID
            DEN = IN + 1 if li == 1 else HID  # denominator column in s
            wlp = cs["WL1P"] if li == 1 else cs["WL2P"]
            sm = cs["SM1"] if li == 1 else cs["SM2"]
            bm = cs["BM1"] if li == 1 else cs["BM2"]
            blv = cs["BL1"] if li == 1 else cs["BL2"]
            invc = bpool.tile([BLK, 1], dt.float32, tag="invc")
            nc.sync.dma_start(out=invc[:], in_=INVC[b])
            invr = bpool.tile([1, BLK], bf, tag="invr")
            nc.sync.dma_start(out=invr[:], in_=INVR[b])
            cf = bpool.tile([BLK, 2 * 128], bf, tag="cf")
            for hh in range(H):
                rec = bpool.tile([BLK, 1], dt.float32, tag=f"rec{hh}")
                nc.vector.reciprocal(rec[:], s_t[hh][:, DEN:DEN + 1])
                f = bpool.tile([BLK, 1], dt.float32, tag=f"f{hh}")
                nc.vector.tensor_scalar(
                    out=f[:], in0=rec[:], scalar1=invc[:], scalar2=0.5,
                    op0=ALU.mult, op1=ALU.mult)
                nc.vector.tensor_scalar(
                    out=cf[:, hh * 128:(hh + 1) * 128], in0=s_t[hh][:, 0:128],
                    scalar1=f[:], scalar2=None, op0=ALU.mult)
            cft = bpool.tile([128, 2 * BLK], bf, tag="cft")
            for hh in range(H):
                nc.sync.dma_start_transpose(
                    out=cft[:, hh * BLK:(hh + 1) * BLK],
                    in_=cf[:, hh * 128:(hh + 1) * 128])
            qt_ps = mixtile()[:, 0:BLK]
            for hh in range(H):
                nc.tensor.matmul(out=qt_ps[:],
                                 lhsT=wlp[0:D, hh * C:(hh + 1) * C],
                                 rhs=cft[0:D, hh * BLK:(hh + 1) * BLK],
                                 start=(hh == 0), stop=False,
                                 skip_group_check=True)
            nc.tensor.matmul(out=qt_ps[:], lhsT=blv[:], rhs=invr[:],
                             start=False, stop=True, skip_group_check=True)
            yt = bpool.tile([C, BLK], bf, tag="yt")
            nc.scalar.activation(yt[:], qt_ps[:], AF.Prelu,
                                 bias=bm[:], scale=sm[:], alpha=0.01)
            if li == 1:
                ysb = bpool.tile([BLK, C], bf, tag="ysb")
                nc.sync.dma_start_transpose(out=ysb[:], in_=yt[:])
                nc.sync.dma_start(out=YS[b * BLK:(b + 1) * BLK, :], in_=ysb[:])
                nc.sync.dma_start(out=YST[:, b * BLK:(b + 1) * BLK], in_=yt[:])
            else:
                o_ps = mixtile()[0:OUT, 0:BLK]
                nc.tensor.matmul(out=o_ps[:], lhsT=cs["FCW"][:],
                                 rhs=yt[:], start=True, stop=True,
                                 skip_group_check=True)
                osb = bpool.tile([OUT, BLK], dt.float32, tag="osb")
                nc.vector.tensor_scalar(out=osb[:], in0=o_ps[:],
                                        scalar1=cs["FCB"][:], scalar2=None,
                                        op0=ALU.add)
                nc.sync.dma_start(out=OUTT[:, b * BLK:(b + 1) * BLK], in_=osb[:])

        def build_xrb(li, b):
            wr = cs["WR1"] if li == 1 else cs["WR2"]
            if li == 1:
                xl_ = bpool.tile([IN, BLK], bf, tag="xrl")
                nc.sync.dma_start(out=xl_[:], in_=HTC[:, b * BLK:(b + 1) * BLK])
                lhs = xl_
            else:
                ytb = bpool.tile([HID, BLK], bf, tag="ytb")
                nc.sync.dma_start(out=ytb[:], in_=YST[:, b * BLK:(b + 1) * BLK])
                lhs = ytb
            xr_ps = mixtile()[:, 0:HC]
            nc.tensor.matmul(out=xr_ps[:], lhsT=lhs[:], rhs=wr[:],
                             start=True, stop=True, skip_group_check=True)
            xrb = bpool.tile([BLK, HC], bf, tag="xrb")
            nc.vector.tensor_copy(xrb[:], xr_ps[:])
            return xrb

        def layer(li):
            D = 128                              # gathered feature cols
            TD = 8 if li == 1 else HID          # transpose rows
            RW = 128 if li == 1 else 132        # r8 width
            AGW = 128 if li == 1 else HID + 1   # agg rhs width (incl den)
            att = cs["ATT1"] if li == 1 else cs["ATT2"]
            gsrc = H5[:, :] if li == 1 else YF[:, :]

            xrb_of = {}
            s_of = {}
            pending = []

            def pass2(sg, r8, tgc8, ex8):
                for j in range(GSG):
                    g = sg * GSG + j
                    b, first, last = gmap[g]
                    if first:
                        sfull = ps.tile([BLK, 512], dt.float32, tag="s",
                                        name=f"s_{li}_{b}")
                        s_of[b] = [sfull[:, hh * 256:hh * 256 + AGW]
                                   for hh in range(H)]
                    for hh in range(H):
                        oh = gpool.tile([GRP, BLK], bf, tag=f"oh{hh}",
                                        name="oh")
                        gi = j * H + hh
                        nc.vector.tensor_scalar(
                            out=oh[:], in0=iota_m[:], scalar1=tgc8[:, j:j + 1],
                            scalar2=ex8[:, gi:gi + 1],
                            op0=ALU.is_equal, op1=ALU.mult)
                        nc.tensor.matmul(
                            out=s_of[b][hh], lhsT=oh[:],
                            rhs=r8[:, j, 0:AGW],
                            start=(first and hh == 0),
                            stop=(last and hh == H - 1),
                            skip_group_check=True)
                    if last:
                        epilogue(li, b, s_of.pop(b))
                        xrb_of.pop(b, None)

            for sg in range(nsg):
                idx8 = spool.tile([GRP, GSG], dt.int32, tag="idx8")
                nc.sync.dma_start(out=idx8[:], in_=IDXT[sg])
                tgc8 = spool.tile([GRP, GSG], dt.float32, tag="tgc8")
                nc.sync.dma_start(out=tgc8[:], in_=TGTC[sg])
                tgbc = spool.tile([BLK, GSG * GRP], bf, tag="tgbc")
                nc.sync.dma_start(
                    out=tgbc[:],
                    in_=TGTR[sg].squeeze().partition_broadcast(BLK))
                ew8 = spool.tile([2, GSG * GRP], bf, tag="ew8")
                nc.sync.dma_start(out=ew8[:], in_=EWT[sg])

                r8 = spool.tile([GRP, GSG, RW], bf, tag=f"r8_{li}", name="r8")
                for jg in range(GSG):
                    nc.gpsimd.indirect_dma_start(
                        out=r8[:, jg, 0:D], out_offset=None, in_=gsrc,
                        in_offset=bass.IndirectOffsetOnAxis(
                            ap=idx8[:, jg:jg + 1], axis=0))
                if li == 1:
                    # overwrite cols 5:8 with (ew, 1, 0) after the gather
                    nc.sync.dma_start(out=r8[:, :, IN:IN + 3], in_=EWC1[sg])
                else:
                    nc.gpsimd.memset(r8[:, :, HID:HID + 1], 1.0)

                ot8 = spool.tile([BLK, GSG * GRP], bf, tag="ot8")
                nc.vector.tensor_scalar(
                    out=ot8[:, :], in0=tgbc[:],
                    scalar1=iota_p[:], scalar2=None, op0=ALU.is_equal)

                alph8 = spool.tile([GRP, GSG * H], dt.float32, tag="alph8")
                ex8 = spool.tile([GRP, GSG * H], dt.float32, tag="ex8")

                xt2s = []
                for j in range(0, GSG, 2):
                    xt_ps = pxt.tile([HID, 2 * GRP], bf, tag="xt", name="xt_ps")
                    for jj in (0, 1):
                        nc.tensor.transpose(
                            out=xt_ps[:, jj * GRP:(jj + 1) * GRP],
                            in_=r8[:, j + jj, 0:128], identity=ident[:])
                    xt2p = gpool.tile([HID, 2 * GRP], bf, tag="xt2", name="xt2")
                    nc.scalar.copy(xt2p[:], xt_ps[:])
                    xt2s.append(xt2p[:, 0:GRP])
                    xt2s.append(xt2p[:, GRP:2 * GRP])
                for j in range(GSG):
                    g = sg * GSG + j
                    b, first, last = gmap[g]
                    if first:
                        xrb_of[b] = build_xrb(li, b)
                    xt2 = xt2s[j]
                    esl = slice(j * GRP, (j + 1) * GRP)
                    t_ps = pt.tile([GRP, HC], dt.float32, tag="t", name="t")
                    if li == 1:
                        nc.tensor.matmul(out=t_ps[:], lhsT=xt2[0:8, :],
                                         rhs=cs["WL1F"][:], start=True,
                                         stop=False, skip_group_check=True)
                        nc.tensor.matmul(out=t_ps[:], lhsT=ot8[:, esl],
                                         rhs=xrb_of[b][:], start=False,
                                         stop=True, skip_group_check=True)
                    else:
                        nc.tensor.matmul(out=t_ps[:], lhsT=xt2[:, :],
                                         rhs=cs["WL2A"][:], start=True,
                                         stop=False, skip_group_check=True)
                        nc.tensor.matmul(out=t_ps[:], lhsT=ot8[:, esl],
                                         rhs=xrb_of[b][:], start=False,
                                         stop=False, skip_group_check=True)
                        nc.tensor.matmul(out=t_ps[:], lhsT=ew8[:, esl],
                                         rhs=cs["WB2"][:], start=False,
                                         stop=True, skip_group_check=True)
                    zl = gpool.tile([GRP, HC], bf, tag="zl")
                    nc.scalar.activation(zl[:], t_ps[:], AF.Prelu, alpha=0.2)
                    za = gpool.tile([GRP, H, C], bf, tag="za")
                    nc.vector.tensor_mul(za[:, :, :], zl[:].rearrange(
                        "p (h c) -> p h c", h=H), att[:].rearrange(
                        "p (h c) -> p h c", h=H))
                    nc.vector.tensor_reduce(
                        out=alph8[:, j * H:(j + 1) * H], in_=za[:, :, :],
                        axis=mybir.AxisListType.X, op=ALU.add)

                nc.scalar.activation(ex8[:], alph8[:], AF.Exp)

                pending.append((sg, r8, tgc8, ex8))
                if len(pending) > 2:
                    pass2(*pending.pop(0))

            for args in pending:
                pass2(*args)
            pending.clear()

        layer(1)
        if cores > 1:
            nc.gpsimd.collective_compute(
                "AllGather", mybir.AluOpType.bypass,
                replica_groups=[list(range(cores))],
                ins=[YS[:, :]], outs=[YF[:, :]])
        else:
            nc.sync.dma_start(out=YF[:, :], in_=YS[:, :])
        layer(2)

    nc.compile()
    return nc


def run(inputs, n, cores, run_sim=False, trace=False):
    h = np.asarray(inputs["h"], np.float32)
    edge_index = np.asarray(inputs["edge_index"])
    edge_weight = np.asarray(inputs["edge_weight"], np.float32)

    per_core, h5p, gmap, bpc, nsg, npad, new2old = _host_prep(
        h, edge_index, edge_weight, n, cores)
    consts = _weights_host(inputs)
    consts["H5"] = h5p

    nc = _build_nc(gmap, bpc, nsg, npad, cores)

    in_maps = []
    for c in range(cores):
        m = dict(consts)
        m.update(per_core[c])
        in_maps.append({k: np.ascontiguousarray(v) for k, v in m.items()})

    if run_sim:
        from concourse import bass_interp
        if cores == 1:
            sim = bass_interp.CoreSim(nc)
            sims = [sim]
        else:
            sim = bass_interp.MultiCoreSim(nc, num_cores=cores)
            sims = list(sim.cores.values())
        for ci, cs_ in enumerate(sims):
            for k, v in in_maps[ci].items():
                cs_.tensor(k)[:] = v
        sim.simulate()
        outs = [np.array(cs_.tensor("OUTT")).T for cs_ in sims]
        res = None
    else:
        from concourse.bass_utils import run_bass_kernel_spmd
        import tempfile
        kw = {}
        if trace:
            kw = dict(trace=True, tmpdir=tempfile.mkdtemp(prefix="basstrace_"))
        res = run_bass_kernel_spmd(nc, in_maps, core_ids=list(range(cores)), **kw)
        outs = [r["OUTT"].T for r in res.results]

    out_perm = np.concatenate(outs, axis=0)  # [npad, OUT] in permuted order
    out = np.empty((n, OUT), np.float32)
    valid = new2old < n
    out[new2old[valid]] = out_perm[valid].astype(np.float32)
    return out, res


def kernel(**inputs):
    out, _ = run(inputs, N, 8)
    return out
